# revision 6
# baseline (speedup 1.0000x reference)
"""Product VQ kernel for Trainium2 (8 NeuronCores, SPMD data-parallel over tokens).

Computes, for z [65536, 256] f32 and codebook [8, 512, 32] f32:
  - all_z_q [65536, 256] f32 (straight-through quantized z)
  - loss scalar f32
  - all_idx [65536, 8] int32 (argmin code per quantizer)

Strategy per core (8192 tokens):
  v[n,k] = -d2 = -(S_z[n,q] + S_e[q,k]) + 2*z.e   computed on PE so that the
  fp32 rounding structure matches the reference ((S_z+S_e) rounded once, then
  one add against the 2*z.e matmul accumulation in PSUM).
  argmax over k per token via DVE max/max_index; zq via indirect-DMA gather
  from the codebook in HBM; loss from the sum of maxima. The straight-through
  epilogue (z + (zq - z)) is exact IEEE elementwise, done on host.
"""

import numpy as np

NQ = 8
KCODES = 512
DSUB = 32
EDIM = 256
NCORES = 8

_CACHE = {}


def _build(ntok: int):
    """Build the Bass program for one core processing `ntok` tokens."""
    import concourse.bacc as bacc
    import concourse.bass as bass
    import concourse.tile as tile
    from concourse import mybir

    f32 = mybir.dt.float32
    i32 = mybir.dt.int32
    u32 = mybir.dt.uint32

    nblk = ntok // 128
    nc = bacc.Bacc("TRN2", target_bir_lowering=False, debug=False)

    # ---- DRAM parameters (inputs) ----
    # zT: z transposed, [256 channels, ntok]
    zt = nc.declare_dram_parameter("zt", [256, ntok], f32, isOutput=False)
    # szk[g, 2*i+r, :]: r=0 -> S_z for quantizer q=4g+i per token; r=1 -> ones
    szk = nc.declare_dram_parameter("szk", [2, 8, ntok], f32, isOutput=False)
    # sek[g, 2*i+r, :]: r=0 -> -1.0s; r=1 -> -S_e for quantizer q=4g+i per code
    sek = nc.declare_dram_parameter("sek", [2, 8, KCODES], f32, isOutput=False)
    # cbt2[g, 32*i+d, k] = 2*codebook[4g+i, k, d]
    cbt2 = nc.declare_dram_parameter("cbt2", [2, 128, KCODES], f32, isOutput=False)
    # flattened codebook rows for the gather: [8*512, 32]
    cbflat = nc.declare_dram_parameter("cbflat", [NQ * KCODES, DSUB], f32, isOutput=False)

    # ---- DRAM outputs ----
    zq_out = nc.declare_dram_parameter("zq", [ntok, EDIM], f32, isOutput=True)
    idx_out = nc.declare_dram_parameter("idx", [ntok, NQ], i32, isOutput=True)
    loss_out = nc.declare_dram_parameter("losspart", [1, 1], f32, isOutput=True)

    with tile.TileContext(nc) as tc:
        with (
            tc.tile_pool(name="big", bufs=1) as big,
            tc.tile_pool(name="consts", bufs=1) as consts,
            tc.tile_pool(name="psum", bufs=7, space="PSUM") as psum,
            tc.tile_pool(name="psum2", bufs=1, space="PSUM") as psum2,
            tc.tile_pool(name="work", bufs=4) as work,
            tc.tile_pool(name="out", bufs=4) as outp,
        ):
            # ---- resident loads ----
            ztile = [big.tile([128, ntok], f32, tag=f"zt{g}", name=f"ztile{g}") for g in range(2)]
            for g in range(2):
                nc.sync.dma_start(out=ztile[g][:, :], in_=zt[128 * g:128 * (g + 1), :])
            szsb = [big.tile([98, ntok], f32, tag=f"sz{g}", name=f"szsb{g}") for g in range(2)]
            sesb = [consts.tile([98, KCODES], f32, tag=f"se{g}", name=f"sesb{g}") for g in range(2)]
            cbsb = [consts.tile([128, KCODES], f32, tag=f"cb{g}", name=f"cbsb{g}") for g in range(2)]
            for g in range(2):
                for i in range(4):
                    nc.sync.dma_start(out=szsb[g][32 * i:32 * i + 2, :],
                                      in_=szk[g, 2 * i:2 * i + 2, :])
                    nc.sync.dma_start(out=sesb[g][32 * i:32 * i + 2, :],
                                      in_=sek[g, 2 * i:2 * i + 2, :])
                nc.sync.dma_start(out=cbsb[g][:, :], in_=cbt2[g, :, :])

            acc = consts.tile([128, NQ], f32)
            nc.vector.memset(acc, 0.0)

            for blk in range(nblk):
                tsl = slice(128 * blk, 128 * (blk + 1))
                vt = {}
                for g in range(2):
                    for i in range(4):
                        q = 4 * g + i
                        v = psum.tile([128, KCODES], f32, tag="v", name=f"v{blk}_{q}")
                        vt[q] = v
                        # psum = -(S_z + S_e)   (single K=2 fp32 matmul)
                        nc.tensor.matmul(
                            out=v[:, :],
                            lhsT=szsb[g][32 * i:32 * i + 2, tsl],
                            rhs=sesb[g][32 * i:32 * i + 2, :],
                            start=True, stop=False,
                            tile_position=(32 * i, 0),
                        )
                        # psum += 2*z.e
                        nc.tensor.matmul(
                            out=v[:, :],
                            lhsT=ztile[g][32 * i:32 * i + 32, tsl],
                            rhs=cbsb[g][32 * i:32 * i + 32, :],
                            start=False, stop=True,
                            tile_position=(32 * i, 0),
                        )

                idxall = outp.tile([128, NQ], i32, tag="idxall")
                gidxf = work.tile([128, NQ], f32, tag="gidxf")
                for q in range(NQ):
                    mx = work.tile([128, 8], f32, tag="mx")
                    ix = work.tile([128, 8], u32, tag="ix")
                    nc.vector.max(out=mx[:, :], in_=vt[q][:, :])
                    nc.vector.max_index(out=ix[:, :], in_max=mx[:, :], in_values=vt[q][:, :])
                    nc.vector.tensor_copy(out=idxall[:, q:q + 1], in_=ix[:, 0:1])
                    nc.vector.tensor_scalar(
                        out=gidxf[:, q:q + 1], in0=ix[:, 0:1],
                        scalar1=float(q * KCODES), scalar2=None,
                        op0=mybir.AluOpType.add,
                    )
                    nc.vector.tensor_add(acc[:, q:q + 1], acc[:, q:q + 1], mx[:, 0:1])

                gidx = work.tile([128, NQ], i32, tag="gidx")
                nc.vector.tensor_copy(out=gidx[:, :], in_=gidxf[:, :])

                zq_sb = outp.tile([128, NQ, DSUB], f32, tag="zq")
                for q in range(NQ):
                    nc.gpsimd.indirect_dma_start(
                        out=zq_sb[:, q, :],
                        out_offset=None,
                        in_=cbflat[:, :],
                        in_offset=bass.IndirectOffsetOnAxis(ap=gidx[:, q:q + 1], axis=0),
                    )
                nc.sync.dma_start(out=zq_out[tsl, :], in_=zq_sb[:, :, :])
                nc.sync.dma_start(out=idx_out[tsl, :], in_=idxall[:, :])

            # ---- loss partial: sum over all (tok, q) of max(v) ----
            accr = work.tile([128, 1], f32)
            nc.vector.tensor_reduce(out=accr[:, :], in_=acc[:, :],
                                    axis=mybir.AxisListType.X, op=mybir.AluOpType.add)
            onesc = consts.tile([128, 1], f32)
            nc.vector.memset(onesc, 1.0)
            lps = psum2.tile([1, 1], f32, tag="lps")
            nc.tensor.matmul(out=lps[:, :], lhsT=onesc[:, :], rhs=accr[:, :],
                             start=True, stop=True)
            lsb = work.tile([1, 1], f32)
            nc.vector.tensor_copy(out=lsb[:, :], in_=lps[:, :])
            nc.sync.dma_start(out=loss_out[:, :], in_=lsb[:, :])

    nc.compile()
    return nc


def _prep_inputs(z: np.ndarray, codebook: np.ndarray, ntok: int, core: int):
    """Build the per-core input map (host-side layout prep only)."""
    lo, hi = core * ntok, (core + 1) * ntok
    zs = z[lo:hi]  # [ntok, 256]
    zt = np.ascontiguousarray(zs.T)  # [256, ntok]

    zc = zs.reshape(ntok, NQ, DSUB)
    sz = np.sum(zc * zc, axis=-1, dtype=np.float32)  # [ntok, 8]
    szk = np.zeros((2, 8, ntok), dtype=np.float32)
    for g in range(2):
        for i in range(4):
            szk[g, 2 * i] = sz[:, 4 * g + i]
            szk[g, 2 * i + 1] = 1.0
    return {"zt": zt, "szk": szk}


def _prep_shared(codebook: np.ndarray):
    se = np.sum(codebook * codebook, axis=-1, dtype=np.float32)  # [8, 512]
    sek = np.zeros((2, 8, KCODES), dtype=np.float32)
    cbt2 = np.zeros((2, 128, KCODES), dtype=np.float32)
    for g in range(2):
        for i in range(4):
            q = 4 * g + i
            sek[g, 2 * i] = -1.0
            sek[g, 2 * i + 1] = -se[q]
            cbt2[g, 32 * i:32 * i + 32, :] = (2.0 * codebook[q]).T
    cbflat = np.ascontiguousarray(codebook.reshape(NQ * KCODES, DSUB))
    return {"sek": sek, "cbt2": cbt2, "cbflat": cbflat}


def run_device(z, codebook, ntok=None, ncores=NCORES, trace=False):
    """Run the device kernel; returns (zq_gathered, idx, loss_scalar, results_obj)."""
    from concourse.bass_utils import run_bass_kernel_spmd

    n = z.shape[0]
    if ntok is None:
        ntok = n // ncores
    assert ntok * ncores == n

    key = ntok
    if key not in _CACHE:
        _CACHE[key] = _build(ntok)
    nc = _CACHE[key]

    shared = _prep_shared(codebook)
    in_maps = []
    for c in range(ncores):
        m = _prep_inputs(z, codebook, ntok, c)
        m.update(shared)
        in_maps.append(m)

    res = run_bass_kernel_spmd(nc, in_maps, list(range(ncores)), trace=trace)
    zq = np.concatenate([res.results[c]["zq"] for c in range(ncores)], axis=0)
    idx = np.concatenate([res.results[c]["idx"] for c in range(ncores)], axis=0)
    vsum = sum(float(res.results[c]["losspart"][0, 0]) for c in range(ncores))
    return zq, idx, vsum, res


def kernel(z: np.ndarray, codebook: np.ndarray):
    z = np.asarray(z, dtype=np.float32)
    codebook = np.asarray(codebook, dtype=np.float32)
    n = z.shape[0]

    zq, idx, vsum, _ = run_device(z, codebook)

    # straight-through estimator epilogue (exact IEEE fp32 elementwise)
    all_z_q = (z + (zq - z)).astype(np.float32)

    # loss = 1.25 * mean(d2_min) / DSUB ; d2_min sum = -sum(max v)
    denom = float(NQ * n * DSUB)
    m = np.float32(-vsum / denom)
    loss = np.float32(np.float32(0.25) * m + m)

    return all_z_q, loss, idx.astype(np.int32)


# revision 10
# speedup vs baseline: 9988.4397x; 9988.4397x over previous
"""Product VQ kernel for Trainium2 (8 NeuronCores, SPMD data-parallel over tokens).

Computes, for z [65536, 256] f32 and codebook [8, 512, 32] f32:
  - all_z_q [65536, 256] f32 (straight-through quantized z)
  - loss scalar f32
  - all_idx [65536, 8] int32 (argmin code per quantizer)

Strategy per core (8192 tokens):
  v[n,k] = -d2 = -(S_z[n,q] + S_e[q,k]) + 2*z.e   computed on PE so that the
  fp32 rounding structure matches the reference ((S_z+S_e) rounded once, then
  one add against the 2*z.e matmul accumulation in PSUM).
  argmax over k per token via DVE max/max_index; zq via indirect-DMA gather
  from the codebook in HBM; loss from the sum of maxima. The straight-through
  epilogue (z + (zq - z)) is exact IEEE elementwise, done on host.
"""

import numpy as np

NQ = 8
KCODES = 512
DSUB = 32
EDIM = 256
NCORES = 8

_CACHE = {}


def _build(ntok: int, tie: str = 'fwd', order: str = 'zfirst', reps: int = 1, zdt: str = 'f32'):
    """Build the Bass program for one core processing `ntok` tokens."""
    import concourse.bacc as bacc
    import concourse.bass as bass
    import concourse.tile as tile
    from concourse import mybir

    f32 = mybir.dt.float32
    i32 = mybir.dt.int32
    u32 = mybir.dt.uint32

    nblk = ntok // 128
    nc = bacc.Bacc("TRN2", target_bir_lowering=False, debug=False)

    # ---- DRAM parameters (inputs) ----
    # zT: z transposed, [256 channels, ntok]
    zt = nc.declare_dram_parameter("zt", [256, ntok], f32, isOutput=False)
    # szk[g, 2*i+r, :]: r=0 -> S_z for quantizer q=4g+i per token; r=1 -> ones
    szk = nc.declare_dram_parameter("szk", [2, 8, ntok], f32, isOutput=False)
    # sek[g, 2*i+r, :]: r=0 -> -1.0s; r=1 -> -S_e for quantizer q=4g+i per code
    sek = nc.declare_dram_parameter("sek", [2, 8, KCODES], f32, isOutput=False)
    # cbt2[g, 32*i+d, k] = 2*codebook[4g+i, k, d]
    cbt2 = nc.declare_dram_parameter("cbt2", [2, 128, KCODES], f32, isOutput=False)
    # flattened codebook rows for the gather: [8*512, 32]
    cbflat = nc.declare_dram_parameter("cbflat", [NQ * KCODES, DSUB], f32, isOutput=False)

    # ---- DRAM outputs ----
    zq_out = nc.declare_dram_parameter("zq", [ntok, EDIM], f32, isOutput=True)
    idx_out = nc.declare_dram_parameter("idx", [ntok, NQ], i32, isOutput=True)
    loss_out = nc.declare_dram_parameter("losspart", [1, 1], f32, isOutput=True)

    with tile.TileContext(nc) as tc:
        with (
            tc.tile_pool(name="big", bufs=1) as big,
            tc.tile_pool(name="consts", bufs=1) as consts,
            tc.tile_pool(name="psum", bufs=7, space="PSUM") as psum,
            tc.tile_pool(name="psum2", bufs=1, space="PSUM") as psum2,
            tc.tile_pool(name="work", bufs=4) as work,
            tc.tile_pool(name="out", bufs=4) as outp,
        ):
            # ---- resident loads ----
            ztile = [big.tile([128, ntok], f32, tag=f"zt{g}", name=f"ztile{g}") for g in range(2)]
            for g in range(2):
                nc.sync.dma_start(out=ztile[g][:, :], in_=zt[128 * g:128 * (g + 1), :])
            szsb = [big.tile([98, ntok], f32, tag=f"sz{g}", name=f"szsb{g}") for g in range(2)]
            sesb = [consts.tile([98, KCODES], f32, tag=f"se{g}", name=f"sesb{g}") for g in range(2)]
            cbsb = [consts.tile([128, KCODES], f32, tag=f"cb{g}", name=f"cbsb{g}") for g in range(2)]
            for g in range(2):
                for i in range(4):
                    nc.sync.dma_start(out=szsb[g][32 * i:32 * i + 2, :],
                                      in_=szk[g, 2 * i:2 * i + 2, :])
                    nc.sync.dma_start(out=sesb[g][32 * i:32 * i + 2, :],
                                      in_=sek[g, 2 * i:2 * i + 2, :])
                nc.sync.dma_start(out=cbsb[g][:, :], in_=cbt2[g, :, :])

            acc = consts.tile([128, NQ], f32)
            nc.vector.memset(acc, 0.0)
            qoffs = consts.tile([128, NQ], f32)
            for q in range(NQ):
                nc.vector.memset(qoffs[:, q:q + 1], float(q * KCODES))

            import contextlib
            loop_cm = tc.For_i(0, reps, 1) if reps > 1 else contextlib.nullcontext()
            with loop_cm:
              for blk in range(nblk):
                tsl = slice(128 * blk, 128 * (blk + 1))
                vt = {}
                for g in range(2):
                    for i in range(4):
                        q = 4 * g + i
                        v = psum.tile([128, KCODES], f32, tag="v", name=f"v{blk}_{q}")
                        vt[q] = v
                        mmS = dict(lhsT=szsb[g][32 * i:32 * i + 2, tsl],
                                   rhs=sesb[g][32 * i:32 * i + 2, :])
                        zl = ztile[g][32 * i:32 * i + 32, tsl]
                        zr = cbsb[g][32 * i:32 * i + 32, :]
                        if zdt == 'f32r':
                            zl = zl.bitcast(mybir.dt.float32r)
                            zr = zr.bitcast(mybir.dt.float32r)
                        mmZ = dict(lhsT=zl, rhs=zr)
                        first, second = (mmS, mmZ) if order == 'sfirst' else (mmZ, mmS)
                        nc.tensor.matmul(out=v[:, :], start=True, stop=False,
                                         tile_position=(32 * i, 0), **first)
                        nc.tensor.matmul(out=v[:, :], start=False, stop=True,
                                         tile_position=(32 * i, 0), **second)

                idxall = outp.tile([128, NQ], i32, tag="idxall")
                gidxf = work.tile([128, NQ], f32, tag="gidxf")
                mxall = work.tile([128, NQ, 8], f32, tag="mxall")
                ixall = work.tile([128, NQ, 8], u32, tag="ixall")
                for q in range(NQ):
                    nc.vector.max(out=mxall[:, q, :], in_=vt[q][:, :])
                    nc.vector.max_index(out=ixall[:, q, :], in_max=mxall[:, q, :],
                                        in_values=vt[q][:, :])
                nc.vector.tensor_copy(out=idxall[:, :], in_=ixall[:, :, 0])
                nc.vector.tensor_tensor(out=gidxf[:, :], in0=ixall[:, :, 0],
                                        in1=qoffs[:, :], op=mybir.AluOpType.add)
                nc.vector.tensor_add(acc[:, :], acc[:, :], mxall[:, :, 0])
                gidx = work.tile([128, NQ], i32, tag="gidx")
                nc.vector.tensor_copy(out=gidx[:, :], in_=gidxf[:, :])

                zq_sb = outp.tile([128, NQ, DSUB], f32, tag="zq")
                for q in range(NQ):
                    nc.gpsimd.indirect_dma_start(
                        out=zq_sb[:, q, :],
                        out_offset=None,
                        in_=cbflat[:, :],
                        in_offset=bass.IndirectOffsetOnAxis(ap=gidx[:, q:q + 1], axis=0),
                    )
                nc.sync.dma_start(out=zq_out[tsl, :], in_=zq_sb[:, :, :])
                nc.sync.dma_start(out=idx_out[tsl, :], in_=idxall[:, :])

            # ---- loss partial: sum over all (tok, q) of max(v) ----
            accr = work.tile([128, 1], f32)
            nc.vector.tensor_reduce(out=accr[:, :], in_=acc[:, :],
                                    axis=mybir.AxisListType.X, op=mybir.AluOpType.add)
            onesc = consts.tile([128, 1], f32)
            nc.vector.memset(onesc, 1.0)
            lps = psum2.tile([1, 1], f32, tag="lps")
            nc.tensor.matmul(out=lps[:, :], lhsT=onesc[:, :], rhs=accr[:, :],
                             start=True, stop=True)
            lsb = work.tile([1, 1], f32)
            nc.vector.tensor_copy(out=lsb[:, :], in_=lps[:, :])
            nc.sync.dma_start(out=loss_out[:, :], in_=lsb[:, :])

    nc.compile()
    return nc


def _prep_inputs(z: np.ndarray, codebook: np.ndarray, ntok: int, core: int):
    """Build the per-core input map (host-side layout prep only)."""
    lo, hi = core * ntok, (core + 1) * ntok
    zs = z[lo:hi]  # [ntok, 256]
    zt = np.ascontiguousarray(zs.T)  # [256, ntok]

    zc = zs.reshape(ntok, NQ, DSUB)
    sz = np.sum(zc * zc, axis=-1, dtype=np.float32)  # [ntok, 8]
    szk = np.zeros((2, 8, ntok), dtype=np.float32)
    for g in range(2):
        for i in range(4):
            szk[g, 2 * i] = sz[:, 4 * g + i]
            szk[g, 2 * i + 1] = 1.0
    return {"zt": zt, "szk": szk}


def _prep_shared(codebook: np.ndarray):
    se = np.sum(codebook * codebook, axis=-1, dtype=np.float32)  # [8, 512]
    sek = np.zeros((2, 8, KCODES), dtype=np.float32)
    cbt2 = np.zeros((2, 128, KCODES), dtype=np.float32)
    for g in range(2):
        for i in range(4):
            q = 4 * g + i
            sek[g, 2 * i] = -1.0
            sek[g, 2 * i + 1] = -se[q]
            cbt2[g, 32 * i:32 * i + 32, :] = (2.0 * codebook[q]).T
    cbflat = np.ascontiguousarray(codebook.reshape(NQ * KCODES, DSUB))
    return {"sek": sek, "cbt2": cbt2, "cbflat": cbflat}


def run_device(z, codebook, ntok=None, ncores=NCORES, trace=False, tie='fwd', order='zfirst', zdt='f32'):
    """Run the device kernel; returns (zq_gathered, idx, loss_scalar, results_obj)."""
    from concourse.bass_utils import run_bass_kernel_spmd

    n = z.shape[0]
    if ntok is None:
        ntok = n // ncores
    assert ntok * ncores == n

    key = (ntok, tie, order, zdt)
    if key not in _CACHE:
        _CACHE[key] = _build(ntok, tie=tie, order=order, zdt=zdt)
    nc = _CACHE[key]

    shared = _prep_shared(codebook)
    in_maps = []
    for c in range(ncores):
        m = _prep_inputs(z, codebook, ntok, c)
        m.update(shared)
        in_maps.append(m)

    res = run_bass_kernel_spmd(nc, in_maps, list(range(ncores)), trace=trace)
    zq = np.concatenate([res.results[c]["zq"] for c in range(ncores)], axis=0)
    idx = np.concatenate([res.results[c]["idx"] for c in range(ncores)], axis=0)
    vsum = sum(float(res.results[c]["losspart"][0, 0]) for c in range(ncores))
    return zq, idx, vsum, res


def kernel(z: np.ndarray, codebook: np.ndarray):
    z = np.asarray(z, dtype=np.float32)
    codebook = np.asarray(codebook, dtype=np.float32)
    n = z.shape[0]

    zq, idx, vsum, _ = run_device(z, codebook)

    # straight-through estimator epilogue (exact IEEE fp32 elementwise)
    all_z_q = (z + (zq - z)).astype(np.float32)

    # loss = 1.25 * mean(d2_min) / DSUB ; d2_min sum = -sum(max v)
    denom = float(NQ * n * DSUB)
    m = np.float32(-vsum / denom)
    loss = np.float32(np.float32(0.25) * m + m)

    return all_z_q, loss, idx.astype(np.int32)
